# revision 4
# baseline (speedup 1.0000x reference)
"""ARAP gradient kernel for 8 TRN2 NeuronCores — v16.

Vertex-sharded: core r owns vertices [12500r, 12500(r+1)) for ALL 8 batches.
Each core builds a bf16 feature table slice [p1(3), t(3), R(9), 1] x 8 batches
(256B per vertex row), AllGathers the full table, then gathers its 200K edges'
neighbor rows as QUAD rows (4 vertices = 1024B per descriptor) so the int16
gather indices cover the whole 102400-row table in ONE pass (baseline needed
4 masked chunk passes = 4x the descriptors). The 4 sub-row selection is folded
into the per-edge weight masks (w at the edge's residue slot, 0 elsewhere) and
accumulated across the 4 residues in PSUM by the PE segment-reduce matmuls.
Gather calls round-robin the 4 SWDGE queues so descriptor-ring drain overlaps
descriptor generation. Final per-vertex combine:

  g_i = aw * (2*W*p2_i - R_i(W*p1_i - S1_i) - SR_i*p1_i + St_i)

where S1 = sum w*p1_j, St = sum w*(R_j p1_j - 2 p2_j), SR = sum w*R_j,
W = sum w (from the constant-1 feature slot).
"""

import numpy as np

B = 8
N = 100000
K = 16
NCORES = 8
VREAL = N // NCORES          # 12500 real vertices per core
VPC = 12800                  # padded vertices per core (= 128*100)
QCOL = 100                   # columns per partition in vertex-major layouts
HCOL = 50                    # half-pass columns (SBUF pressure)
TROWS = NCORES * VPC         # 102400 global table rows
QROWS = TROWS // 4           # 25600 quad rows (int16-addressable)
F = 128                      # 8 batches * 16 bf16 feature slots
MQ = 32                      # gather columns per tile
NT = (VPC * K // 128) // MQ  # 50 tiles (1600 columns total)
NIDX_T = MQ * 128            # 4096 indices per tile
NSPLIT = 4                   # dma_gathers per tile (ring cap + queue spread)
MQS = MQ // NSPLIT
NIDX_S = NIDX_T // NSPLIT

_cache = {}


def _build():
    from concourse import bass, bacc, mybir
    from concourse.tile import TileContext

    nc = bacc.Bacc(None, num_swdge_queues=4)
    dt = mybir.dt

    xyz1_p = nc.declare_dram_parameter("xyz1s", [B, VPC, 3], dt.float32, isOutput=False)
    xyz2_p = nc.declare_dram_parameter("xyz2s", [B, VPC, 3], dt.float32, isOutput=False)
    rot_p = nc.declare_dram_parameter("rots", [B, VPC, 9], dt.float32, isOutput=False)
    idx_p = nc.declare_dram_parameter("idxw", [NT, NSPLIT, 128, NIDX_S // 16], dt.int16, isOutput=False)
    wq_p = nc.declare_dram_parameter("wts", [NT, 128, 4, MQ], dt.bfloat16, isOutput=False)
    ones_p = nc.declare_dram_parameter("ones16", [128, 8], dt.bfloat16, isOutput=False)
    aw_p = nc.declare_dram_parameter("aw", [128, 1], dt.float32, isOutput=False)
    g_p = nc.declare_dram_parameter("g", [B, VPC, 3], dt.float32, isOutput=True)

    myT = nc.dram_tensor("myT", [VPC, F], dt.bfloat16)
    T_all = nc.dram_tensor("T_all", [TROWS, F], dt.bfloat16, addr_space="Shared")
    S_dram = nc.dram_tensor("S_dram", [VPC, F], dt.float32)

    with TileContext(nc) as tc:
        with (
            tc.tile_pool(name="sbuf", bufs=2) as pool,
            tc.tile_pool(name="feat", bufs=1) as fpool,
            tc.tile_pool(name="gat", bufs=2) as gpool,
            tc.tile_pool(name="wgp", bufs=2) as wpool,
            tc.tile_pool(name="gq3", bufs=3) as gq3pool,
            tc.tile_pool(name="stp", bufs=1) as stpool,
            tc.tile_pool(name="psum", bufs=1, space="PSUM") as ppool,
        ):
            ones_t = fpool.tile([128, 8], dt.bfloat16, tag="ones")
            nc.sync.dma_start(out=ones_t[:], in_=ones_p[:, :])
            aw_t = fpool.tile([128, 1], dt.float32, tag="aw")
            nc.sync.dma_start(out=aw_t[:], in_=aw_p[:, :])

            # ---- Phase A: feature table slice (two half-passes) -------------
            for h in range(2):
                c0 = HCOL * h
                FS = fpool.tile([128, HCOL, F], dt.bfloat16, tag="FS")
                nc.vector.memset(FS[:, :, :], 1.0)
                for b in range(B):
                    p1 = pool.tile([128, HCOL, 3], dt.float32, tag="p1")
                    p2 = pool.tile([128, HCOL, 3], dt.float32, tag="p2")
                    R = pool.tile([128, HCOL, 9], dt.float32, tag="R")
                    nc.sync.dma_start(out=p1[:], in_=xyz1_p[b].rearrange("(p q) c -> p q c", p=128)[:, c0:c0 + HCOL, :])
                    nc.sync.dma_start(out=p2[:], in_=xyz2_p[b].rearrange("(p q) c -> p q c", p=128)[:, c0:c0 + HCOL, :])
                    nc.sync.dma_start(out=R[:], in_=rot_p[b].rearrange("(p q) c -> p q c", p=128)[:, c0:c0 + HCOL, :])
                    fo = b * 16
                    # p1 -> slots 0:3
                    nc.vector.tensor_copy(out=FS[:, :, fo + 0 : fo + 3], in_=p1[:, :, :])
                    # t = R @ p1 - 2*p2 -> slots 3:6
                    for a in range(3):
                        acc = pool.tile([128, HCOL], dt.float32, tag="acc")
                        tmp = pool.tile([128, HCOL], dt.float32, tag="tmp")
                        nc.vector.tensor_tensor(out=acc[:], in0=R[:, :, 3 * a], in1=p1[:, :, 0], op=mybir.AluOpType.mult)
                        nc.vector.tensor_tensor(out=tmp[:], in0=R[:, :, 3 * a + 1], in1=p1[:, :, 1], op=mybir.AluOpType.mult)
                        nc.vector.tensor_tensor(out=acc[:], in0=acc[:], in1=tmp[:], op=mybir.AluOpType.add)
                        nc.vector.tensor_tensor(out=tmp[:], in0=R[:, :, 3 * a + 2], in1=p1[:, :, 2], op=mybir.AluOpType.mult)
                        nc.vector.tensor_tensor(out=acc[:], in0=acc[:], in1=tmp[:], op=mybir.AluOpType.add)
                        nc.vector.tensor_scalar_mul(out=tmp[:], in0=p2[:, :, a], scalar1=-2.0)
                        nc.vector.tensor_tensor(out=FS[:, :, fo + 3 + a], in0=acc[:], in1=tmp[:], op=mybir.AluOpType.add)
                    # R -> slots 6:15  (slot 15 stays 1.0 from memset)
                    nc.vector.tensor_copy(out=FS[:, :, fo + 6 : fo + 15], in_=R[:, :, :])
                nc.sync.dma_start(
                    out=myT[6400 * h : 6400 * (h + 1)].rearrange("(p j) f -> p j f", p=128),
                    in_=FS[:, :, :],
                )
                # ---- Phase B: AllGather this half while computing the next --
                nc.gpsimd.collective_compute(
                    "AllGather",
                    mybir.AluOpType.bypass,
                    replica_groups=[list(range(NCORES))],
                    ins=[myT[6400 * h : 6400 * (h + 1)]],
                    outs=[T_all[51200 * h : 51200 * (h + 1)]],
                )

            # ---- Phase D helper: combine one (half, batch) v-block ----------
            # Loads are issued ~2 tiles before the compute block runs so the
            # in-order DVE stream never stalls on them.
            def phase_d_load(hh, b):
                v0b = 6400 * hh
                p1 = pool.tile([128, HCOL, 3], dt.float32, tag="p1d")
                p2 = pool.tile([128, HCOL, 3], dt.float32, tag="p2d")
                R = pool.tile([128, HCOL, 9], dt.float32, tag="Rd")
                nc.sync.dma_start(out=p1[:], in_=xyz1_p[b][v0b : v0b + 6400].rearrange("(p j) c -> p j c", p=128))
                nc.sync.dma_start(out=p2[:], in_=xyz2_p[b][v0b : v0b + 6400].rearrange("(p j) c -> p j c", p=128))
                nc.sync.dma_start(out=R[:], in_=rot_p[b][v0b : v0b + 6400].rearrange("(p j) c -> p j c", p=128))
                return p1, p2, R

            def phase_d_compute(hh, b, S, loaded, use_pool):
                teng = nc.gpsimd if use_pool else nc.vector
                v0b = 6400 * hh
                p1, p2, R = loaded
                fo = b * 16
                W = S[:, :, fo + 15]
                awb = aw_t[:, :].to_broadcast([128, HCOL])
                gout = pool.tile([128, HCOL, 3], dt.float32, tag="gout")
                u = pool.tile([128, HCOL, 3], dt.float32, tag="u")
                for a in range(3):
                    tmp = pool.tile([128, HCOL], dt.float32, tag="tmpd")
                    nc.vector.tensor_tensor(out=tmp[:], in0=W, in1=p1[:, :, a], op=mybir.AluOpType.mult)
                    nc.vector.tensor_tensor(out=u[:, :, a], in0=tmp[:], in1=S[:, :, fo + a], op=mybir.AluOpType.subtract)
                for a in range(3):
                    acc = pool.tile([128, HCOL], dt.float32, tag="accd")
                    tmp = pool.tile([128, HCOL], dt.float32, tag="tmpd")
                    nc.vector.tensor_tensor(out=acc[:], in0=R[:, :, 3 * a], in1=u[:, :, 0], op=mybir.AluOpType.mult)
                    nc.vector.tensor_tensor(out=tmp[:], in0=R[:, :, 3 * a + 1], in1=u[:, :, 1], op=mybir.AluOpType.mult)
                    nc.vector.tensor_tensor(out=acc[:], in0=acc[:], in1=tmp[:], op=mybir.AluOpType.add)
                    nc.vector.tensor_tensor(out=tmp[:], in0=R[:, :, 3 * a + 2], in1=u[:, :, 2], op=mybir.AluOpType.mult)
                    nc.vector.tensor_tensor(out=acc[:], in0=acc[:], in1=tmp[:], op=mybir.AluOpType.add)
                    for j in range(3):
                        teng.tensor_tensor(out=tmp[:], in0=S[:, :, fo + 6 + 3 * a + j], in1=p1[:, :, j], op=mybir.AluOpType.mult)
                        nc.vector.tensor_tensor(out=acc[:], in0=acc[:], in1=tmp[:], op=mybir.AluOpType.add)
                    teng.tensor_tensor(out=tmp[:], in0=W, in1=p2[:, :, a], op=mybir.AluOpType.mult)
                    nc.vector.tensor_scalar_mul(out=tmp[:], in0=tmp[:], scalar1=2.0)
                    nc.vector.tensor_tensor(out=tmp[:], in0=tmp[:], in1=acc[:], op=mybir.AluOpType.subtract)
                    nc.vector.tensor_tensor(out=tmp[:], in0=tmp[:], in1=S[:, :, fo + 3 + a], op=mybir.AluOpType.add)
                    nc.vector.tensor_tensor(out=gout[:, :, a], in0=tmp[:], in1=awb, op=mybir.AluOpType.mult)
                nc.sync.dma_start(out=g_p[b][v0b : v0b + 6400].rearrange("(p j) c -> p j c", p=128), in_=gout[:])

            # ---- Phase C: quad-row gather + weighted segment reduce ---------
            T_all_q = T_all.rearrange("(q m) f -> q (m f)", m=4)
            S0 = None
            d_loaded = None
            for t in range(NT):
                if t == 25:
                    S0 = fpool.tile([128, HCOL, F], dt.float32, tag="S0")
                    nc.sync.dma_start(out=S0[:], in_=S_dram[0:6400].rearrange("(p j) f -> p j f", p=128))
                if t >= 25 and (t - 25) % 3 == 0 and (t - 25) // 3 < 8:
                    d_loaded = phase_d_load(0, (t - 25) // 3)
                if t >= 27 and (t - 27) % 3 == 0 and (t - 27) // 3 < 8:
                    phase_d_compute(0, (t - 27) // 3, S0, d_loaded, use_pool=False)
                idx_t = gpool.tile([128, NSPLIT, NIDX_S // 16], dt.int16, tag="idx")
                nc.sync.dma_start(out=idx_t[:], in_=idx_p[t].rearrange("sp p q -> p sp q"))
                wq_t = gpool.tile([128, 4, MQ, 1], dt.bfloat16, tag="wq")
                nc.sync.dma_start(out=wq_t[:, :, :, 0], in_=wq_p[t])
                gq = gq3pool.tile([128, MQ, 4 * F], dt.bfloat16, tag="gq")
                for sp in range(NSPLIT):
                    nc.gpsimd.dma_gather(
                        out_ap=gq[:, MQS * sp : MQS * (sp + 1), :],
                        in_ap=T_all_q[:, :],
                        idxs_ap=idx_t[:, sp, :],
                        num_idxs=NIDX_S,
                        num_idxs_reg=NIDX_S,
                        elem_size=4 * F,
                        queue_num=sp,
                    )
                pss = [ppool.tile([8, 512], dt.float32, name=f"ps{qq}_{t}", tag=f"ps{qq}") for qq in range(8)]
                St = stpool.tile([8, 8, 4, 128], dt.float32, tag="St")
                for m in range(4):
                    wg = wpool.tile([128, MQ, F], dt.bfloat16, tag="wg")
                    nc.vector.tensor_tensor(
                        out=wg[:, :, :],
                        in0=gq[:, :, F * m : F * (m + 1)],
                        in1=wq_t[:, m, :, :].to_broadcast([128, MQ, F]),
                        op=mybir.AluOpType.mult,
                    )
                    for qq in range(8):
                        nc.tensor.matmul(
                            out=pss[qq][:, :],
                            lhsT=ones_t[:, :],
                            rhs=wg[:, 4 * qq : 4 * qq + 4, :],
                            start=(m == 0),
                            stop=(m == 3),
                        )
                        if m == 3:
                            # copy each bank out as soon as its accumulation
                            # closes; overlaps the remaining stop-matmuls
                            nc.scalar.copy(out=St[:, qq, :, :], in_=pss[qq][:, :])
                # S row = 256*t + 32*qq + 8*q2 + s
                dst = S_dram[256 * t : 256 * (t + 1)].rearrange("(qq q2 s) f -> s qq q2 f", qq=8, q2=4)
                nc.sync.dma_start(out=dst, in_=St[:, :, :, :])

            # ---- Phase D: remaining half ------------------------------------
            S1 = fpool.tile([128, HCOL, F], dt.float32, tag="S0")
            nc.sync.dma_start(out=S1[:], in_=S_dram[6400:12800].rearrange("(p j) f -> p j f", p=128))
            for b in range(B):
                ld = phase_d_load(1, b)
                phase_d_compute(1, b, S1, ld, use_pool=True)
    nc.compile()
    return nc


def _host_prep(xyz1, xyz2, neighborList, numNeighbors, accnumNeighbors, weightMatrix, rotations, arapWeight):
    """Index/layout-only preprocessing. Returns per-core input maps."""
    nbr = np.asarray(neighborList).astype(np.int64)
    wm = np.asarray(weightMatrix).astype(np.float32)
    # global table row of vertex j: half-major layout so each half of the
    # table can be AllGathered as one contiguous block while phase A still
    # computes the other half.  row = 51200*h + 6400*core + 50*p + j where
    # vertex-local loc = 100*p + q, h = q//50, j = q%50.
    _c = nbr // VREAL
    _loc = nbr % VREAL
    _p = _loc // 100
    _q = _loc % 100
    rows = 51200 * (_q // 50) + 6400 * _c + 50 * _p + (_q % 50)

    ones16 = np.zeros((128, 8), np.float32)
    for p in range(128):
        ones16[p, p // 16] = 1.0
    import jax.numpy as jnp
    ones16 = np.asarray(jnp.asarray(ones16, jnp.bfloat16))

    in_maps = []
    for r in range(NCORES):
        v0 = r * VREAL
        xyz1s = np.zeros((B, VPC, 3), np.float32)
        xyz2s = np.zeros((B, VPC, 3), np.float32)
        rots = np.zeros((B, VPC, 9), np.float32)
        xyz1s[:, :VREAL] = xyz1[:, v0 : v0 + VREAL]
        xyz2s[:, :VREAL] = xyz2[:, v0 : v0 + VREAL]
        rots[:, :VREAL] = np.asarray(rotations[:, v0 : v0 + VREAL]).reshape(B, VREAL, 9)

        # Edge slot (t, col, p): vertex v = 256t + 32*(col//4) + 8*(col%4) + p//16,
        # neighbor slot k = p % 16. Gather position within tile = col*128 + p.
        t_i = np.arange(NT)[:, None, None]
        col = np.arange(MQ)[None, :, None]
        p_i = np.arange(128)[None, None, :]
        v_loc = 256 * t_i + 32 * (col // 4) + 8 * (col % 4) + p_i // 16   # [NT, MQ, 128]
        kk = p_i % 16
        valid = v_loc < VREAL
        vg = np.where(valid, v_loc + v0, 0)
        e_ids = vg * K + kk
        rws = np.where(valid, rows[e_ids], 0)          # [NT, MQ, 128]
        wvals = np.where(valid, wm[e_ids], 0.0)

        qidx = (rws // 4).astype(np.int16)             # quad row, < 25600
        mres = rws % 4

        # wrapped-16 int16 indices, replicated to 128 partitions:
        # position i -> lane i%16, flat i//16
        flat = qidx.reshape(NT, NSPLIT, NIDX_S)        # [t, sp, i]
        relw = flat.reshape(NT, NSPLIT, NIDX_S // 16, 16)
        idxw = np.tile(np.transpose(relw, (0, 1, 3, 2)), (1, 1, 8, 1))

        # residue-masked weights: wq[t, p, mm, col] = w if edge residue == mm
        wqf = np.zeros((NT, 128, 4, MQ), np.float32)
        for mm in range(4):
            wqf[:, :, mm, :] = np.transpose(np.where(mres == mm, wvals, 0.0), (0, 2, 1))
        wq = np.asarray(jnp.asarray(wqf, jnp.bfloat16))

        in_maps.append({
            "xyz1s": xyz1s, "xyz2s": xyz2s, "rots": rots,
            "idxw": idxw, "wts": wq, "ones16": ones16,
            "aw": np.full((128, 1), np.float32(arapWeight)),
        })
    return in_maps


def _execute(in_maps, trace=False, **kw):
    from concourse.bass_utils import run_bass_kernel_spmd
    if "nc" not in _cache:
        _cache["nc"] = _build()
    return run_bass_kernel_spmd(_cache["nc"], in_maps, list(range(NCORES)), trace=trace, **kw)


def kernel(**inputs):
    in_maps = _host_prep(**inputs)
    res = _execute(in_maps)
    out = np.concatenate([res.results[r]["g"][:, :VREAL] for r in range(NCORES)], axis=1)
    return out.astype(np.float32)
